# revision 1
# baseline (speedup 1.0000x reference)
"""Causal core-attention kernel for Trainium2, 8-core SPMD.

Problem: q,k,v [2048, 2, 16, 128] fp32, causal mask, softmax(QK^T/sqrt(128)) @ V,
output [2048, 2, 2048] fp32.

Sharding: the 32 (batch, head) pairs are split 4-per-core across 8 NeuronCores.
No cross-core communication.

Per-core algorithm (per (b,h) pair), flash-style but without max subtraction
(scores have unit variance so exp never overflows; the reference's -10000 mask
fill underflows to exactly 0 in fp32, so masked positions contribute 0):

  for each s-tile (512 queries): key blocks (128 keys) are packed causally
  tight into 3-bank PSUM tiles (each key block only computes the s-extent at
  128 granularity that the mask allows, bank-fitted so no matmul output
  crosses a PSUM bank):
    MM1 (TensorE, bf16):  S^T[128 t, ext*128 s] = K_blk^T.T @ Q^T  (contract d)
    exp (ScalarE):        P^T = exp(S^T * 1/sqrt(128)) -> bf16, one instruction
                          per contiguous run of a PSUM tile
    mask (VectorE):       P^T[:, 128 cols] *= pattern   (diagonal blocks only)
    MM2 (TensorE, bf16):  acc[128 s, 130] += P_sub^T.T @ [V_blk | 1 | 0]
                          (col 128 accumulates the softmax denominator)
  normalize (VectorE):    out = acc[:, :128] * (1 / acc[:, 128])

  bf16 (not fp16) is essential: fp16-subnormal exp outputs ran 95x slower on
  real TRN2 silicon.

Latency shaping: per-pair K/Q/V are three separate DMAs (MM1 waits only on
K+Q); the inputs pool holds 3 pairs so prefetch has slack; the mask-pattern
DMA is issued after pair 0's K+Q; the last pair walks its s-tiles in
descending order so the program ends on the smallest tile; output DMAs go
through HWDGE (sync) rather than SWDGE.
"""

import math

import ml_dtypes
import numpy as np

import concourse.bass as bass
import concourse.mybir as mybir
import concourse.tile as tile
from concourse.bass_utils import run_bass_kernel_spmd

SQ, B, NH, HN = 2048, 2, 16, 128
N_CORES = 8
N_PAIRS = (B * NH) // N_CORES  # 4 (b,h) pairs per core
TB = 128  # key-block size (t)
VW = HN + 2  # V padded with a ones column (denominator) + a zero column (even width)
SCALE = 1.0 / math.sqrt(HN)
NBANK = 3  # PSUM banks per score tile
BANK_SLOTS = 4  # 128-col slots per PSUM bank (512 fp32)

FULL, EMPTY, PARTIAL = 0, 1, 2

_last_results = None  # BassKernelResults of the most recent kernel() call

f16 = mybir.dt.bfloat16
f32 = mybir.dt.float32
_np16 = ml_dtypes.bfloat16


def _classify_mask(allowed: np.ndarray, sq: int, sblk: int):
    """Host-side mask analysis. allowed[s, t] True where attention is permitted.

    Builds, per s-tile, a list of packed PSUM-tile descriptors. Each needed
    key block j contributes a segment covering the 128-granular s-extent
    [off_u, end_u) where the mask is not all-False; segments are best-fit
    packed into 4-slot banks, NBANK banks per PSUM tile, so every MM1 output
    stays inside one bank. Within a tile, fuller banks are placed first so
    exp can run over contiguous runs.
    """
    nsb = sq // TB  # 128-row s blocks (sigma)
    ntb = sq // TB  # 128-col t blocks (j)
    nsub = sblk // TB  # s sub-blocks per s-tile
    nst = sq // sblk  # s tiles

    st = allowed.reshape(nsb, TB, ntb, TB)
    blk_all = st.all(axis=(1, 3))  # [sigma, j]
    blk_any = st.any(axis=(1, 3))
    status = np.where(blk_all, FULL, np.where(blk_any, PARTIAL, EMPTY))

    patterns: list[np.ndarray] = []
    pat_index: dict[bytes, int] = {}

    def pat_id(sig: int, j: int) -> int:
        # [t, s] orientation to match P^T
        pat = np.ascontiguousarray(
            allowed[sig * TB : (sig + 1) * TB, j * TB : (j + 1) * TB].T
        ).astype(_np16)
        key = pat.tobytes()
        if key not in pat_index:
            pat_index[key] = len(patterns)
            patterns.append(pat)
        return pat_index[key]

    # first/last non-empty j per 128-row s block (for PSUM start/stop flags)
    first_j = np.full(nsb, -1, np.int64)
    last_j = np.full(nsb, -1, np.int64)
    for g in range(nsb):
        js = [j for j in range(ntb) if status[g, j] != EMPTY]
        if js:
            first_j[g], last_j[g] = js[0], js[-1]

    def finalize_bins(i, bins):
        """bins: list of lists of atoms (j, off_u, ext). Returns ptile
        descriptors. MM2 start/stop flags are derived from emission order:
        per acc bank (u pair) the first-emitted matmul is the bank leader's
        start=True (clears has_written for the whole bank); each sigma's
        last-emitted matmul carries stop=True. Any bin order is legal."""
        ptiles = []
        all_mm2 = []  # (ptile_idx, pos, u, col, j)
        for atoms in bins:
            banks = [[] for _ in range(NBANK)]
            free = [BANK_SLOTS] * NBANK
            for (j, off_u, ext) in sorted(atoms, key=lambda a: -a[2]):
                cands = [b for b in range(NBANK) if free[b] >= ext]
                if not cands:
                    return None
                b = min(cands, key=lambda x: free[x])
                banks[b].append((j, off_u, ext, BANK_SLOTS - free[b]))
                free[b] -= ext
            order = sorted(
                range(NBANK), key=lambda b: BANK_SLOTS - free[b], reverse=True
            )
            segs = []  # (j, col0, off_u, ext)
            runs = []  # (col0, ncols) contiguous used regions for exp
            muls = []  # (col, pattern_idx)
            mm2 = []  # (u, col, j)
            run_start, run_end = None, None
            for pos, b in enumerate(order):
                used = BANK_SLOTS - free[b]
                if used == 0:
                    continue
                base = pos * BANK_SLOTS * TB
                if run_start is None:
                    run_start, run_end = base, base + used * TB
                elif base == run_end:
                    run_end = base + used * TB
                else:
                    runs.append((run_start, run_end - run_start))
                    run_start, run_end = base, base + used * TB
                for (j, off_u, ext, slot) in banks[b]:
                    col0 = base + slot * TB
                    segs.append((j, col0, off_u, ext))
                    for u in range(off_u, off_u + ext):
                        sig = i * nsub + u
                        stt = status[sig, j]
                        if stt == EMPTY:
                            continue
                        col = col0 + (u - off_u) * TB
                        if stt == PARTIAL:
                            muls.append((col, pat_id(sig, j)))
                        mm2.append((u, col, j))
            if run_start is not None:
                runs.append((run_start, run_end - run_start))
            mm2.sort(key=lambda t: (t[2], t[0]))  # j-major, ascending u
            for pos, ent in enumerate(mm2):
                all_mm2.append((len(ptiles), pos, *ent))
            ptiles.append(dict(segs=segs, runs=runs, muls=muls, mm2=mm2))
        bank_seen: set[int] = set()
        sig_last: dict[int, tuple] = {}
        flags: dict[tuple, list] = {}
        for (g, pos, u, col, j) in all_mm2:
            start = u // 2 not in bank_seen
            bank_seen.add(u // 2)
            flags[(g, pos)] = [start, False]
            sig_last[u] = (g, pos)
        for key in sig_last.values():
            flags[key][1] = True
        for g, pt in enumerate(ptiles):
            pt["mm2"] = [
                (u, col, j, *flags[(g, pos)])
                for pos, (u, col, j) in enumerate(pt["mm2"])
            ]
        return ptiles

    def pack_tile(i, cap, mode):
        atoms = []
        for j in range(ntb):
            present = [u for u in range(nsub) if status[i * nsub + u, j] != EMPTY]
            if not present:
                continue
            off_u, end_u = present[0], present[-1] + 1
            atoms.append((j, off_u, end_u - off_u))
        if not atoms:
            return []
        total = sum(a[2] for a in atoms)
        if mode == "single":
            res = finalize_bins(i, [[a] for a in atoms])
            if res is not None:
                return res
        elif mode == "balance":
            # LPT into ceil(total/cap) bins for near-equal exp sizes
            nt = -(-total // cap)
            bins = [[] for _ in range(nt)]
            loads = [0] * nt
            for a in sorted(atoms, key=lambda a: -a[2]):
                cands = [b for b in range(nt) if loads[b] + a[2] <= cap]
                if not cands:
                    bins.append([])
                    loads.append(0)
                    cands = [len(bins) - 1]
                b = min(cands, key=lambda x: loads[x])
                bins[b].append(a)
                loads[b] += a[2]
            bins = [b for b in bins if b]
            bins.sort(key=lambda b: min(a[0] for a in b))
            res = finalize_bins(i, bins)
            if res is not None:
                return res
        # fallback: sequential first-fit (j-ordered)
        bins, load = [[]], 0
        for a in atoms:
            if load + a[2] > cap:
                bins.append([])
                load = 0
            bins[-1].append(a)
            load += a[2]
        res = finalize_bins(i, bins)
        assert res is not None
        return res

    # tiles: balanced full-size packing; tiles_fine: one key block per exp,
    # used for the program's very last s-tile so the tail after the final
    # exp instruction is minimal
    tiles = [pack_tile(i, NBANK * BANK_SLOTS, "balance") for i in range(nst)]
    tiles_fine = [pack_tile(i, BANK_SLOTS, "single") for i in range(nst)]

    pats_host = None
    if patterns:
        # [TB partitions, n_pat, TB] contiguous for a clean DMA
        pats_host = np.ascontiguousarray(np.stack(patterns, axis=0).transpose(1, 0, 2))

    return dict(
        status=status,
        tiles=tiles,
        tiles_fine=tiles_fine,
        first_j=first_j,
        last_j=last_j,
        pats_host=pats_host,
        nst=nst,
        nsub=nsub,
        nsb=nsb,
    )


def _split_multiwaits(nc):
    """The walrus build in this container supports exactly one sync-wait per
    instruction (NEURON_ISA_TPB_EVENTS has a single wait slot) and does not
    split multi-wait instructions itself. Tile emits instructions with several
    waits; lower each extra wait onto a same-engine NoOp carrier inserted
    immediately before the instruction (identical stall point, no reordering).
    """
    n_new = 0
    for blk in nc.m.functions[0].blocks:
        insts = blk.instructions
        i = 0
        while i < len(insts):
            ins = insts[i]
            si = ins.sync_info
            if si is not None and len(si.on_wait) > 1:
                waits = list(si.on_wait)
                carriers = []
                for w in waits[:-1]:
                    n_new += 1
                    carriers.append(
                        mybir.InstNoOp(
                            name=f"I-swsplit-{n_new}",
                            engine=ins.engine,
                            ins=[],
                            outs=[],
                            sync_info=mybir.SyncInfo(on_wait=[w], on_update=[]),
                        )
                    )
                ins.sync_info = mybir.SyncInfo(
                    on_wait=[waits[-1]], on_update=list(si.on_update)
                )
                insts[i:i] = carriers
                i += len(carriers)
            i += 1
    return n_new


def _build_program(sched, sq: int, sblk: int, n_pairs: int, repeat: int = 1):
    n_pat = 0 if sched["pats_host"] is None else sched["pats_host"].shape[1]
    first_j = sched["first_j"]
    last_j = sched["last_j"]
    tiles = sched["tiles"]
    tiles_fine = sched["tiles_fine"]
    nst, nsub = sched["nst"], sched["nsub"]
    nblk = sq // TB

    nc = bass.Bass(
        "TRN2", target_bir_lowering=False, debug=False, num_devices=N_CORES
    )
    W = 2 * sq + nblk * VW  # concatenated [K^T | Q^T | V_aug] width per pair
    qkv = nc.dram_tensor("qkv", [n_pairs, TB, W], f16, kind="ExternalInput").ap()
    pats = None
    if n_pat:
        pats = nc.dram_tensor(
            "pats", [TB, n_pat, TB], f16, kind="ExternalInput"
        ).ap()
    out = nc.dram_tensor(
        "out_ctx", [n_pairs, nst, TB, nsub * HN], f32, kind="ExternalOutput"
    ).ap()

    with tile.TileContext(nc) as tc:
        with (
            tc.tile_pool(name="inputs", bufs=3) as inputs,
            tc.tile_pool(name="consts", bufs=1) as consts,
            tc.tile_pool(name="ptp", bufs=4) as ptp,
            tc.tile_pool(name="outp", bufs=4) as outp,
            tc.tile_pool(name="stp", bufs=2, space="PSUM") as stp,
            tc.tile_pool(name="accp", bufs=1, space="PSUM") as accp,
        ):
            pat_t = None
            seq = [(rep, p) for rep in range(repeat) for p in range(n_pairs)]
            bufs: dict[int, tuple] = {}

            def issue_input_dma(idx: int):
                nonlocal pat_t
                rep, p = seq[idx]
                k_t = inputs.tile([TB, sq], f16, tag="k", name=f"k_{idx}")
                q_t = inputs.tile([TB, sq], f16, tag="q", name=f"q_{idx}")
                v_t = inputs.tile([TB, nblk * VW], f16, tag="v", name=f"v_{idx}")
                if idx == 0:
                    # latency-split: the first score tile needs only the low
                    # K/Q columns; the mask patterns must beat the first
                    # diagonal-block multiply; V must beat the first MM2.
                    # A one-column slice lands first to feed PE warm-up
                    # matmuls (the PE clock ramps only under sustained
                    # activity; warming during the DMA saves the ramp later)
                    head = min(8 * TB, sq)
                    nc.sync.dma_start(out=k_t[:, 0:head], in_=qkv[p][:, 0:head])
                    nc.sync.dma_start(
                        out=q_t[:, 0:head], in_=qkv[p][:, sq : sq + head]
                    )
                    if n_pat:
                        pat_t = consts.tile([TB, n_pat, TB], f16)
                        nc.sync.dma_start(out=pat_t, in_=pats)
                    nc.sync.dma_start(out=k_t[:, head:sq], in_=qkv[p][:, head:sq])
                    nc.sync.dma_start(
                        out=q_t[:, head:sq], in_=qkv[p][:, sq + head : 2 * sq]
                    )
                    nc.sync.dma_start(out=v_t, in_=qkv[p][:, 2 * sq :])
                else:
                    nc.sync.dma_start(out=k_t, in_=qkv[p][:, 0:sq])
                    nc.sync.dma_start(out=q_t, in_=qkv[p][:, sq : 2 * sq])
                    nc.sync.dma_start(out=v_t, in_=qkv[p][:, 2 * sq :])
                bufs[idx] = (k_t, q_t, v_t.rearrange("p (j c) -> p j c", c=VW))

            # pending MM2 batches: batch of PSUM tile g is emitted during the
            # processing of PSUM tile g+2, so the PE FIFO order is
            # MM1(g), MM2(g-2), MM1(g+1), MM2(g-1), ... — MM2(g-2) is always
            # sem-ready (its exp finished a full period ago) and MM1(g+1)
            # completes well before exp(g) ends, keeping ScalarE saturated
            pending = []

            def flush_one():
                mm2s, fin = pending.pop(0)
                for (out_ap, lhsT, rhs, start, stop) in mm2s:
                    nc.tensor.matmul(
                        out_ap, lhsT=lhsT, rhs=rhs, start=start, stop=stop,
                        skip_group_check=True,
                    )
                if fin is not None:
                    fin()

            def flush_all():
                while pending:
                    flush_one()

            issue_input_dma(0)
            if len(seq) > 1:
                issue_input_dma(1)

            for idx, (rep, p) in enumerate(seq):
                if idx + 2 < len(seq):
                    issue_input_dma(idx + 2)
                k_t, q_t, v_t = bufs.pop(idx)

                tile_order = (
                    list(range(nst - 1, -1, -1))
                    if idx == len(seq) - 1
                    else list(range(nst))
                )
                for ti, i in enumerate(tile_order):
                    is_final = idx == len(seq) - 1 and ti == len(tile_order) - 1
                    descs = tiles_fine[i] if is_final else tiles[i]
                    # Accumulators packed 2-per-PSUM-bank. start=True clears
                    # has_written for the WHOLE bank, so only the bank leader
                    # (first-emitted matmul of the pair) carries start=True;
                    # the partner's first matmul lands on cleared bits and
                    # overwrites-then-sets per element, which begins its
                    # accumulation chain.
                    acc_pairs = [
                        accp.tile(
                            [TB, 2, VW], f32, tag=f"accpair{h}", name=f"acc_{p}_{i}_{h}"
                        )
                        for h in range((nsub + 1) // 2)
                    ]
                    accs = [acc_pairs[u // 2][:, u % 2, :] for u in range(nsub)]
                    for g0, pt_desc in enumerate(descs):
                        st_t = stp.tile(
                            [TB, NBANK * BANK_SLOTS * TB],
                            f32,
                            tag="st",
                            name=f"st_{p}_{i}_{g0}",
                        )
                        for (j, col0, off_u, ext) in pt_desc["segs"]:
                            nc.tensor.matmul(
                                st_t[:, col0 : col0 + ext * TB],
                                lhsT=k_t[:, j * TB : (j + 1) * TB],
                                rhs=q_t[
                                    :,
                                    i * sblk
                                    + off_u * TB : i * sblk
                                    + (off_u + ext) * TB,
                                ],
                                start=True,
                                stop=True,
                            )
                        pt = ptp.tile(
                            [TB, NBANK * BANK_SLOTS * TB],
                            f16,
                            tag="pt",
                            name=f"pt_{p}_{i}_{g0}",
                        )
                        for (col0, ncols) in pt_desc["runs"]:
                            nc.scalar.activation(
                                pt[:, col0 : col0 + ncols],
                                st_t[:, col0 : col0 + ncols],
                                mybir.ActivationFunctionType.Exp,
                                scale=SCALE,
                            )
                        for (col, pi) in pt_desc["muls"]:
                            nc.vector.tensor_mul(
                                pt[:, col : col + TB],
                                pt[:, col : col + TB],
                                pat_t[:, pi, :],
                            )
                        while len(pending) >= 2:
                            flush_one()
                        mm2s = []
                        for (u, col, j, start, stop) in pt_desc["mm2"]:
                            mm2s.append(
                                (
                                    accs[u],
                                    pt[:, col : col + TB],
                                    v_t[:, j, :],
                                    start,
                                    stop,
                                )
                            )
                        fin = None
                        if g0 == len(descs) - 1:

                            def fin(
                                p=p, i=i, accs=accs, acc_pairs=acc_pairs,
                                is_final=is_final,
                            ):
                                ot = outp.tile(
                                    [TB, nsub, HN], f32, tag="ot", name=f"ot_{p}_{i}"
                                )
                                rec = outp.tile(
                                    [TB, (nsub + 1) // 2, 2],
                                    f32,
                                    tag="rec",
                                    name=f"rec_{p}_{i}",
                                )
                                if is_final:
                                    # per-u chains so each sub-block's
                                    # normalize runs as soon as its own
                                    # accumulation stops; ship all but the
                                    # last sub-block early
                                    for u in range(nsub):
                                        ru = rec[:, u // 2, u % 2 : u % 2 + 1]
                                        nc.vector.reciprocal(
                                            ru, accs[u][:, HN : HN + 1]
                                        )
                                        nc.vector.tensor_scalar_mul(
                                            ot[:, u, :], accs[u][:, 0:HN], ru
                                        )
                                        if u == nsub - 2:
                                            nc.sync.dma_start(
                                                out=out[p, i][:, 0 : (u + 1) * HN],
                                                in_=ot[:, 0 : u + 1, :],
                                            )
                                    nc.sync.dma_start(
                                        out=out[p, i][:, (nsub - 1) * HN :],
                                        in_=ot[:, nsub - 1 :, :],
                                    )
                                    return
                                for h in range((nsub + 1) // 2):
                                    nc.vector.reciprocal(
                                        rec[:, h, :], acc_pairs[h][:, :, HN]
                                    )
                                for u in range(nsub):
                                    nc.vector.tensor_scalar_mul(
                                        ot[:, u, :],
                                        accs[u][:, 0:HN],
                                        rec[:, u // 2, u % 2 : u % 2 + 1],
                                    )
                                nc.sync.dma_start(out=out[p, i], in_=ot)

                        pending.append((mm2s, fin))
            flush_all()
    _split_multiwaits(nc)
    return nc


def _prep_inputs(query_layer, key_layer, value_layer, sq, n_pairs_total):
    """Transpose + cast on host into one DMA-friendly concatenated layout:
    per pair, [128 partitions, 2*sq + nblk*VW] = [K^T | Q^T | V_aug]."""
    nblk = sq // TB
    W = 2 * sq + nblk * VW
    qkv = np.empty((n_pairs_total, TB, W), _np16)
    # [s, b, nh, hn] -> [pair, hn, s] (d-major, contraction on partitions)
    qkv[:, :, 0:sq] = key_layer.transpose(1, 2, 3, 0).reshape(n_pairs_total, HN, sq)
    qkv[:, :, sq : 2 * sq] = query_layer.transpose(1, 2, 3, 0).reshape(
        n_pairs_total, HN, sq
    )
    # [s, b, nh, hn] -> [pair, s, hn] -> augmented, t-in-block on partitions
    v = value_layer.transpose(1, 2, 0, 3).reshape(n_pairs_total, sq, HN)
    va = np.zeros((n_pairs_total, sq, VW), _np16)
    va[:, :, :HN] = v
    va[:, :, HN] = 1.0
    qkv[:, :, 2 * sq :] = (
        va.reshape(n_pairs_total, nblk, TB, VW)
        .transpose(0, 2, 1, 3)
        .reshape(n_pairs_total, TB, nblk * VW)
    )
    return qkv


def kernel(query_layer, key_layer, value_layer, attention_mask):
    sq = query_layer.shape[0]
    assert query_layer.shape == (sq, B, NH, HN)
    sblk = 512
    n_pairs_total = B * NH

    allowed = ~np.asarray(attention_mask).reshape(sq, sq)
    sched = _classify_mask(allowed, sq, sblk)
    nc = _build_program(sched, sq, sblk, N_PAIRS)

    qkv = _prep_inputs(query_layer, key_layer, value_layer, sq, n_pairs_total)

    in_maps = []
    for c in range(N_CORES):
        m = {"qkv": np.ascontiguousarray(qkv[c * N_PAIRS : (c + 1) * N_PAIRS])}
        if sched["pats_host"] is not None:
            m["pats"] = sched["pats_host"]
        in_maps.append(m)

    import os

    trace = os.environ.get("ATTN_TRACE", "0") == "1"
    res = run_bass_kernel_spmd(
        nc, in_maps, core_ids=list(range(N_CORES)), trace=trace
    )
    global _last_results
    _last_results = res

    out = np.empty((sq, B, NH * HN), np.float32)
    nst, nsub = sched["nst"], sched["nsub"]
    for c in range(N_CORES):
        arr = res.results[c]["out_ctx"]  # [N_PAIRS, nst, TB, nsub*HN]
        for p in range(N_PAIRS):
            b, h = divmod(c * N_PAIRS + p, NH)
            # [nst, TB(r), nsub(u), HN] -> s = i*sblk + u*TB + r
            o = arr[p].reshape(nst, TB, nsub, HN).transpose(0, 2, 1, 3)
            out[:, b, h * HN : (h + 1) * HN] = o.reshape(sq, HN)
    return out


# ---------------------------------------------------------------------------
# Timing utilities (dev-only; the graded path is kernel() above).
# The axon NTFF profiling hook is unavailable in this container, so we measure
# device time by wall-clocking a persistent jitted executable and differencing
# two programs that repeat the compute R1 vs R2 times (constant dispatch/RPC
# overhead cancels).
# ---------------------------------------------------------------------------


def _make_runner(nc, in_maps):
    import jax
    import concourse.mybir as _mybir
    from concourse.bass2jax import (
        _bass_exec_p,
        install_neuronx_cc_hook,
        partition_id_tensor,
    )
    from jax.experimental.shard_map import shard_map
    from jax.sharding import Mesh, NamedSharding, PartitionSpec

    install_neuronx_cc_hook()
    n_cores = len(in_maps)
    partition_name = nc.partition_id_tensor.name if nc.partition_id_tensor else None
    in_names, out_names, out_avals, zero_outs = [], [], [], []
    for alloc in nc.m.functions[0].allocations:
        if not isinstance(alloc, mybir.MemoryLocationSet):
            continue
        name = alloc.memorylocations[0].name
        if alloc.kind == "ExternalInput":
            if name != partition_name:
                in_names.append(name)
        elif alloc.kind == "ExternalOutput":
            out_names.append(name)
            shape = tuple(alloc.tensor_shape)
            dtype = _mybir.dt.np(alloc.dtype)
            out_avals.append(jax.core.ShapedArray(shape, dtype))
            zero_outs.append(np.zeros(shape, dtype))
    n_params = len(in_names)
    all_in_names = in_names + out_names
    if partition_name is not None:
        all_in_names.append(partition_name)

    def _body(*args):
        operands = list(args)
        if partition_name is not None:
            operands.append(partition_id_tensor())
        outs = _bass_exec_p.bind(
            *operands,
            out_avals=tuple(out_avals),
            in_names=tuple(all_in_names),
            out_names=tuple(out_names),
            lowering_input_output_aliases=(),
            sim_require_finite=True,
            sim_require_nnan=True,
            nc=nc,
        )
        return tuple(outs)

    devices = jax.devices()[:n_cores]
    mesh = Mesh(np.asarray(devices), ("core",))
    spec = PartitionSpec("core")
    sharded = jax.jit(
        shard_map(
            _body,
            mesh=mesh,
            in_specs=(spec,) * (n_params + len(out_names)),
            out_specs=(spec,) * len(out_names),
            check_rep=False,
        ),
        keep_unused=True,
    )
    sh = NamedSharding(mesh, spec)
    dev_in = [
        jax.device_put(
            np.concatenate([in_maps[c][n] for c in range(n_cores)], axis=0), sh
        )
        for n in in_names
    ]
    dev_zero = [
        jax.device_put(np.zeros((n_cores * z.shape[0], *z.shape[1:]), z.dtype), sh)
        for z in zero_outs
    ]

    def run():
        return jax.block_until_ready(sharded(*dev_in, *dev_zero))

    return run


def measure_exec_ns(inputs, r1=2, r2=12, iters=12):
    import time

    sq = inputs["query_layer"].shape[0]
    sblk = 512
    allowed = ~np.asarray(inputs["attention_mask"]).reshape(sq, sq)
    sched = _classify_mask(allowed, sq, sblk)
    qkv = _prep_inputs(
        inputs["query_layer"], inputs["key_layer"], inputs["value_layer"], sq, B * NH
    )
    in_maps = []
    for c in range(N_CORES):
        m = {"qkv": np.ascontiguousarray(qkv[c * N_PAIRS : (c + 1) * N_PAIRS])}
        if sched["pats_host"] is not None:
            m["pats"] = sched["pats_host"]
        in_maps.append(m)

    walls = {}
    for r in (r1, r2):
        nc = _build_program(sched, sq, sblk, N_PAIRS, repeat=r)
        run = _make_runner(nc, in_maps)
        run()  # compile + warm
        best = float("inf")
        for _ in range(iters):
            t0 = time.perf_counter()
            run()
            best = min(best, time.perf_counter() - t0)
        walls[r] = best
        print(f"repeat={r}: best wall {best * 1e6:.1f} us")
    per_rep_s = (walls[r2] - walls[r1]) / (r2 - r1)
    return per_rep_s * 1e9

